# revision 1
# baseline (speedup 1.0000x reference)
"""Trainium2 Bass kernel for nn_EquiformerV2Conv (gnn_message_passing).

Mathematical basis: the per-edge rotation matrices R are orthogonal, and the
reference applies R, a channel-mixing linear (acting on the channel index
only), then R^T.  The rotations cancel exactly:
    msg = ew * [ h0[src] @ W0 / sqrt(64) | per-xyz h1[src] @ W1 / sqrt(32) ]
so the network reduces to
    G   = segment_sum(x[src], dst)                  (memory-bound gather/scatter)
    ew  = mean_e sigmoid(rbf(|pos[src]-pos[dst]|) @ pw + pb)   (global scalar)
    agg = ew * [ G0 @ W0/8 | per-xyz G1 @ W1/sqrt(32) ]
    out = [ silu(layernorm(agg0)) | agg1 ]
(validated numerically: rel err ~2e-6 vs the full rotation pipeline).

Distribution: nodes are bin-packed into 51 windows per core x 8 cores with a
fixed per-window edge capacity (512 edges with src<25000 + 512 with
src>=25000), so all cores run one SPMD program.  Each core gathers x rows for
its edges (bf16 hi/lo pair, 768B rows, dma_gather) and scatter-adds them into
per-window PSUM via one-hot matmuls.  ew partial sums are AllReduce'd.
"""
import os
import numpy as np
import ml_dtypes

bf16 = ml_dtypes.bfloat16
f32 = np.float32

# problem constants
N = 50000
E = 400000
SC, VC, DIM, NB = 64, 32, 160, 64
CUTOFF, EPS = 5.0, 1e-5

# distribution constants
P = 128            # slots per block / window slot capacity
W = 51             # windows per core
CAPA = 512         # per-window capacity for A edges (src < SPLIT)
CAPB = 512         # per-window capacity for B edges
SPLIT = 25000
NCORES = 8
BLKW_A = CAPA // P          # 4 blocks per window (A)
BLKW_B = CAPB // P
NBLK_A = W * BLKW_A         # 204
NBLK_B = W * BLKW_B         # 204
NBLK = NBLK_A + NBLK_B      # 408
SLOTS = NBLK * P            # 52224
GW = 3                      # windows per gather call
GBLK_A = GW * BLKW_A        # 12 blocks per A gather call
GBLK_B = GW * BLKW_B
ZSLAB = 12                  # blocks per z-phase slab (408 = 34*12)
ELEM = 384                  # bf16 elements per xp row (768 B)
OUTROWS = W * P             # 6528


# ---------------------------------------------------------------- host side

def _pack_nodes(src, dst):
    isA = src < SPLIT
    degA = np.bincount(dst[isA], minlength=N)
    degB = np.bincount(dst[~isA], minlength=N)
    order = np.argsort(-(degA + degB), kind="stable")
    nbins = NCORES * W
    binA = np.zeros(nbins, np.int64)
    binB = np.zeros(nbins, np.int64)
    binC = np.zeros(nbins, np.int64)
    node2win = np.full(N, -1, np.int64)
    node2slot = np.full(N, -1, np.int64)
    start = 0
    for n in order:
        a, b = degA[n], degB[n]
        for k in range(nbins):
            w = (start + k) % nbins
            if binA[w] + a <= CAPA and binB[w] + b <= CAPB and binC[w] < P:
                node2win[n] = w
                node2slot[n] = binC[w]
                binA[w] += a
                binB[w] += b
                binC[w] += 1
                start = (w + 1) % nbins
                break
        else:
            raise RuntimeError(f"window packing failed at node {n}")
    return node2win, node2slot


def _stage(x, pos, src, dst):
    """Build all per-core device input arrays."""
    node2win, node2slot = _pack_nodes(src, dst)
    win_core = node2win % NCORES
    win_local = node2win // NCORES

    # xp rows: [xhi bf16 160 | xlo bf16 160 | pad 64] with l1 cols j-major
    perm = np.arange(DIM)
    l1 = np.arange(SC, DIM)
    cc = (l1 - SC) // 3
    jj = (l1 - SC) % 3
    perm[SC + 32 * jj + cc] = l1
    xr = x[:, perm]
    xhi = xr.astype(bf16)
    xlo = (xr - xhi.astype(f32)).astype(bf16)
    xp = np.zeros((N, ELEM), bf16)
    xp[:, :160] = xhi
    xp[:, 160:320] = xlo

    e_core = win_core[dst]
    e_wl = win_local[dst]
    e_slot = node2slot[dst]
    e_isB = (src >= SPLIT).astype(np.int64)

    ps = pos[src].astype(bf16)
    pd = pos[dst].astype(bf16)

    # slot assignment: group edges by (core, half, window); cumcount in group
    key = ((e_core * 2 + e_isB) * W + e_wl)
    order = np.argsort(key, kind="stable")
    ks = key[order]
    grp_start = np.searchsorted(ks, np.arange(2 * NCORES * W))
    grp_end = np.searchsorted(ks, np.arange(2 * NCORES * W), side="right")
    within = np.arange(E) - grp_start[ks]
    # slot index within the core
    kb = ks % (2 * W)                 # (half, window) combined, per core
    half = kb // W
    wl = kb % W
    slot_sorted = np.where(half == 0,
                           wl * CAPA + within,
                           CAPA * W + wl * CAPB + within)
    e_sorted = order

    cores = []
    for r in range(NCORES):
        cores.append(dict(
            gidx=np.zeros(SLOTS, np.int16),
            dstw=np.full(SLOTS, 255.0, f32),
            mask=np.zeros(SLOTS, bf16),
            eb=np.zeros((SLOTS, 8), bf16),
        ))
    core_sorted = e_core[e_sorted]
    for r in range(NCORES):
        sel = core_sorted == r
        es = e_sorted[sel]
        sl = slot_sorted[sel]
        cd = cores[r]
        gi = np.where(e_isB[es] == 1, src[es] - SPLIT, src[es])
        cd["gidx"][sl] = gi.astype(np.int16)
        cd["dstw"][sl] = e_slot[es].astype(f32)
        cd["mask"][sl] = bf16(1.0)
        cd["eb"][sl, 0:3] = ps[es]
        cd["eb"][sl, 3:6] = pd[es]

    # reshape to device layouts
    ins = []
    for r in range(NCORES):
        cd = cores[r]
        gidx = cd["gidx"].reshape(SLOTS // 16, 16).T.copy()   # [16, S/16]
        gidx = np.tile(gidx, (8, 1))                          # [128, S/16]
        dstw = cd["dstw"].reshape(NBLK, P).T.copy()           # [128, NBLK]
        mask = cd["mask"].reshape(NBLK, P).T.copy()
        eb = cd["eb"].reshape(NBLK, P, 8).transpose(1, 0, 2).reshape(P, NBLK * 8).copy()
        ins.append(dict(gidx=gidx, dstw=dstw, maskb=mask, eb=eb))

    meta = dict(node2win=node2win, node2slot=node2slot,
                win_core=win_core, win_local=win_local)
    return xp, ins, meta


# ---------------------------------------------------------------- device side

_PROG = None


def _build_program():
    stage = os.environ.get("KSTAGE", "all")
    has_z = stage in ("z", "all")
    has_wmix = stage in ("wmix", "z", "all")
    has_cc = stage == "all"
    import concourse.bacc as bacc
    import concourse.tile as tile
    import concourse.bass as bass
    from concourse import mybir, library_config, bass_isa

    dt = mybir.dt
    Alu = mybir.AluOpType
    Act = mybir.ActivationFunctionType

    nc = bacc.Bacc("TRN2", target_bir_lowering=False, debug=False,
                   num_devices=NCORES)

    xp_d = nc.dram_tensor("xp", [N, ELEM], dt.bfloat16, kind="ExternalInput")
    gidx_d = nc.dram_tensor("gidx", [P, SLOTS // 16], dt.int16, kind="ExternalInput")
    dstw_d = nc.dram_tensor("dstw", [P, NBLK], dt.float32, kind="ExternalInput")
    maskb_d = nc.dram_tensor("maskb", [P, NBLK], dt.bfloat16, kind="ExternalInput")
    eb_d = nc.dram_tensor("eb", [P, NBLK * 8], dt.bfloat16, kind="ExternalInput")
    iota_d = nc.dram_tensor("iota", [P, P], dt.float32, kind="ExternalInput")
    ident_d = nc.dram_tensor("ident", [P, P], dt.float32, kind="ExternalInput")
    w0_d = nc.dram_tensor("w0", [SC, SC], dt.float32, kind="ExternalInput")
    w1_d = nc.dram_tensor("w1", [VC, VC], dt.float32, kind="ExternalInput")
    smalls_d = {}
    for nm in ("cent", "wid", "pwv", "gam", "bet"):
        smalls_d[nm] = nc.dram_tensor(nm, [1, 64], dt.float32, kind="ExternalInput")
    pb_d = nc.dram_tensor("pbv", [1, 1], dt.float32, kind="ExternalInput")
    out_d = nc.dram_tensor("out", [OUTROWS, DIM], dt.float32, kind="ExternalOutput")
    dbg_d = None
    if stage != "all":
        dbg_d = nc.dram_tensor("dbg", [P, 4, DIM], dt.float32, kind="ExternalOutput")
        dbgs_d = nc.dram_tensor("dbgs", [P, 4, P], dt.float32, kind="ExternalOutput")
        dbgx_d = nc.dram_tensor("dbgx", [P, 4, ELEM], dt.float32, kind="ExternalOutput")
        dbgc_d = nc.dram_tensor("dbgc", [P, 2 * P], dt.float32, kind="ExternalOutput")
        dbgsb_d = nc.dram_tensor("dbgsb", [P, 4, P], dt.float32, kind="ExternalOutput")
        dbgxb_d = nc.dram_tensor("dbgxb", [P, 4, ELEM], dt.float32, kind="ExternalOutput")

    s0 = float(1.0 / np.sqrt(SC) / E)
    s1 = float(1.0 / np.sqrt(VC) / E)

    with tile.TileContext(nc, num_cores=NCORES) as tc:
        import contextlib
        with contextlib.ExitStack() as ctx:
            consts = ctx.enter_context(tc.tile_pool(name="consts", bufs=1))
            gbuf = ctx.enter_context(tc.tile_pool(name="gbuf", bufs=1))
            gather = ctx.enter_context(tc.tile_pool(name="gather", bufs=2))
            spool = ctx.enter_context(tc.tile_pool(name="spool", bufs=4))
            zpool = ctx.enter_context(tc.tile_pool(name="zpool", bufs=4))
            trop = ctx.enter_context(tc.tile_pool(name="trop", bufs=2))
            psum = ctx.enter_context(tc.tile_pool(name="psum", bufs=2, space="PSUM"))
            psum1 = ctx.enter_context(tc.tile_pool(name="psum1", bufs=1, space="PSUM"))
            dram = ctx.enter_context(tc.tile_pool(name="dram", bufs=1, space="DRAM"))

            nc.gpsimd.load_library(library_config.mlp)

            # ---- constant loads
            iota = consts.tile([P, P], dt.float32)
            nc.sync.dma_start(out=iota[:], in_=iota_d[:])
            ident = consts.tile([P, P], dt.float32)
            nc.sync.dma_start(out=ident[:], in_=ident_d[:])
            gidx = consts.tile([P, SLOTS // 16], dt.int16)
            nc.sync.dma_start(out=gidx[:], in_=gidx_d[:])
            dstw = consts.tile([P, NBLK], dt.float32)
            nc.sync.dma_start(out=dstw[:], in_=dstw_d[:])
            maskb = consts.tile([P, NBLK], dt.bfloat16)
            nc.sync.dma_start(out=maskb[:], in_=maskb_d[:])
            eb = consts.tile([P, NBLK, 8], dt.bfloat16)
            nc.sync.dma_start(out=eb[:], in_=eb_d[:])
            w0sb = consts.tile([SC, SC], dt.float32)
            nc.sync.dma_start(out=w0sb[:], in_=w0_d[:])
            w1sb = consts.tile([VC, VC], dt.float32)
            nc.sync.dma_start(out=w1sb[:], in_=w1_d[:])

            smalls = {}
            for nm, d in smalls_d.items():
                t = consts.tile([1, 64], dt.float32, tag=f"sm_{nm}")
                nc.sync.dma_start(out=t[:], in_=d[:])
                b = consts.tile([P, 64], dt.float32, tag=f"bc_{nm}")
                nc.gpsimd.partition_broadcast(out_ap=b[:], in_ap=t[:], channels=P)
                smalls[nm] = b
            pbt = consts.tile([1, 1], dt.float32)
            nc.sync.dma_start(out=pbt[:], in_=pb_d[:])
            pbb = consts.tile([P, 1], dt.float32)
            nc.gpsimd.partition_broadcast(out_ap=pbb[:], in_ap=pbt[:], channels=P)
            invw = consts.tile([P, 64], dt.float32)
            nc.vector.reciprocal(out=invw[:], in_=smalls["wid"][:])

            # ---- z phase: per-edge distance -> rbf -> sigmoid -> sum
            if not has_z:
                ewb = gbuf.tile([P, 1], dt.float32)
                nc.vector.memset(ewb[:], 1.0)
            else:
                d2b = gbuf.tile([P, NBLK], dt.float32)
                db = gbuf.tile([P, NBLK], dt.float32)
                zdot = gbuf.tile([P, NBLK], dt.float32)
                nslab = NBLK // ZSLAB
                for s in range(nslab):
                    sl = slice(s * ZSLAB, (s + 1) * ZSLAB)
                    dif = zpool.tile([P, ZSLAB, 3], dt.float32, tag="dif")
                    nc.vector.tensor_tensor(out=dif[:], in0=eb[:, sl, 0:3],
                                            in1=eb[:, sl, 3:6], op=Alu.subtract)
                    sq = zpool.tile([P, ZSLAB, 3], dt.float32, tag="sq")
                    nc.vector.tensor_tensor(out=sq[:], in0=dif[:], in1=dif[:],
                                            op=Alu.mult)
                    nc.vector.tensor_reduce(out=d2b[:, sl], in_=sq[:],
                                            axis=mybir.AxisListType.X, op=Alu.add)
                nc.scalar.activation(out=db[:], in_=d2b[:], func=Act.Sqrt)
                for s in range(nslab):
                    sl = slice(s * ZSLAB, (s + 1) * ZSLAB)
                    dbc = db[:, sl].unsqueeze(2).to_broadcast([P, ZSLAB, 64])
                    t1 = zpool.tile([P, ZSLAB, 64], dt.float32, tag="zt")
                    nc.vector.tensor_tensor(
                        out=t1[:], in0=dbc,
                        in1=smalls["cent"][:].unsqueeze(1).to_broadcast([P, ZSLAB, 64]),
                        op=Alu.subtract)
                    t2 = zpool.tile([P, ZSLAB, 64], dt.float32, tag="zt")
                    nc.vector.tensor_tensor(
                        out=t2[:], in0=t1[:],
                        in1=invw[:].unsqueeze(1).to_broadcast([P, ZSLAB, 64]),
                        op=Alu.mult)
                    t3 = zpool.tile([P, ZSLAB, 64], dt.float32, tag="zt")
                    nc.scalar.activation(out=t3[:], in_=t2[:], func=Act.Square)
                    t4 = zpool.tile([P, ZSLAB, 64], dt.float32, tag="zt")
                    nc.scalar.activation(out=t4[:], in_=t3[:], func=Act.Exp, scale=-0.5)
                    t5 = zpool.tile([P, ZSLAB, 64], dt.float32, tag="zt")
                    nc.vector.tensor_tensor(
                        out=t5[:], in0=t4[:],
                        in1=smalls["pwv"][:].unsqueeze(1).to_broadcast([P, ZSLAB, 64]),
                        op=Alu.mult)
                    nc.vector.tensor_reduce(out=zdot[:, sl], in_=t5[:],
                                            axis=mybir.AxisListType.X, op=Alu.add)
                # cutoff envelope, sigmoid, masked sum
                halfpi = consts.tile([P, 1], dt.float32)
                nc.vector.memset(halfpi[:], float(np.pi / 2))
                dcl = gbuf.tile([P, NBLK], dt.float32)
                nc.vector.tensor_scalar(out=dcl[:], in0=db[:],
                                        scalar1=float(CUTOFF), scalar2=None,
                                        op0=Alu.min)
                cutb = gbuf.tile([P, NBLK], dt.float32)
                # cos(pi*d/5) = sin(pi/2 - pi*d/5); argument in [-pi/2, pi/2]
                nc.scalar.activation(out=cutb[:], in_=dcl[:], func=Act.Sin,
                                     scale=float(-np.pi / CUTOFF),
                                     bias=halfpi[:, 0:1])
                cut2 = gbuf.tile([P, NBLK], dt.float32)
                nc.vector.tensor_scalar(out=cut2[:], in0=cutb[:], scalar1=0.5,
                                        scalar2=0.5, op0=Alu.mult, op1=Alu.add)
                mle = gbuf.tile([P, NBLK], dt.float32)
                nc.vector.tensor_scalar(out=mle[:], in0=db[:], scalar1=float(CUTOFF),
                                        scalar2=None, op0=Alu.is_le)
                cutf = gbuf.tile([P, NBLK], dt.float32)
                nc.vector.tensor_tensor(out=cutf[:], in0=cut2[:], in1=mle[:], op=Alu.mult)
                ub = gbuf.tile([P, NBLK], dt.float32)
                nc.vector.tensor_tensor(out=ub[:], in0=zdot[:], in1=cutf[:], op=Alu.mult)
                zv = gbuf.tile([P, NBLK], dt.float32)
                nc.scalar.activation(out=zv[:], in_=ub[:], func=Act.Sigmoid,
                                     bias=pbb[:, 0:1])
                junk = gbuf.tile([P, NBLK], dt.float32)
                nc.vector.tensor_tensor(out=junk[:], in0=zv[:], in1=maskb[:],
                                        op=Alu.mult)
                zsum = gbuf.tile([P, 1], dt.float32)
                nc.vector.tensor_reduce(out=zsum[:], in_=junk[:],
                                        axis=mybir.AxisListType.X, op=Alu.add)
                zred = gbuf.tile([P, 1], dt.float32)
                nc.gpsimd.partition_all_reduce(out_ap=zred[:], in_ap=zsum[:],
                                               channels=P,
                                               reduce_op=bass_isa.ReduceOp.add)
                z8 = gbuf.tile([1, 8], dt.float32)
                nc.vector.tensor_copy(out=z8[:], in_=zred[0:1, 0:1].to_broadcast([1, 8]))
                ewsb = gbuf.tile([1, 8], dt.float32)
                if has_cc:
                    arin = dram.tile([1, 8], dt.float32)
                    arout = dram.tile([1, 8], dt.float32)
                    nc.sync.dma_start(out=arin[:], in_=z8[:])
                    nc.gpsimd.collective_compute(
                        "AllReduce", Alu.add, replica_groups=[list(range(NCORES))],
                        ins=[arin.opt()], outs=[arout.opt()])
                    nc.sync.dma_start(out=ewsb[:], in_=arout[:])
                else:
                    nc.vector.tensor_copy(out=ewsb[:], in_=z8[:])
            if has_z:
                ewb = gbuf.tile([P, 1], dt.float32)
                nc.gpsimd.partition_broadcast(out_ap=ewb[:], in_ap=ewsb[:, 0:1], channels=P)
            ewc0 = gbuf.tile([P, 1], dt.float32)
            nc.vector.tensor_scalar(out=ewc0[:], in0=ewb[:], scalar1=s0,
                                    scalar2=None, op0=Alu.mult)
            ewc1 = gbuf.tile([P, 1], dt.float32)
            nc.vector.tensor_scalar(out=ewc1[:], in0=ewb[:], scalar1=s1,
                                    scalar2=None, op0=Alu.mult)

            # ---- gather + scatter + W-mix
            if dbg_d is not None:
                cf = spool.tile([P, 2 * P], dt.float32, tag="cf")
                nc.vector.tensor_copy(out=cf[:, 0:P], in_=iota[:])
                nc.vector.tensor_copy(out=cf[:, P:2 * P], in_=dstw[:, 0:P])
                nc.sync.dma_start(out=dbgc_d[:], in_=cf[:])
            obuf = gbuf.tile([P, W, DIM], dt.float32)
            ncalls = NBLK_A // GBLK_A      # 17
            for kc in range(ncalls):
                xga = gather.tile([P, GBLK_A, ELEM], dt.bfloat16, tag="xga")
                a0 = kc * GBLK_A
                nc.gpsimd.dma_gather(
                    xga[:], xp_d[0:SPLIT, :],
                    gidx[:, a0 * 8:(a0 + GBLK_A) * 8],
                    GBLK_A * P, GBLK_A * P, ELEM, single_packet=False)
                xgb = gather.tile([P, GBLK_B, ELEM], dt.bfloat16, tag="xgb")
                b0 = kc * GBLK_B
                nc.gpsimd.dma_gather(
                    xgb[:], xp_d[SPLIT:N, :],
                    gidx[:, (NBLK_A + b0) * 8:(NBLK_A + b0 + GBLK_B) * 8],
                    GBLK_B * P, GBLK_B * P, ELEM, single_packet=False)
                for wi in range(GW):
                    w = kc * GW + wi
                    gps = psum.tile([P, DIM], dt.float32, tag="gps")
                    nmm = (BLKW_A + BLKW_B) * 2
                    mi = 0
                    for i in range(BLKW_A):
                        blk = w * BLKW_A + i
                        S = spool.tile([P, P], dt.bfloat16, tag="S")
                        nc.vector.tensor_scalar(
                            out=S[:], in0=iota[:], scalar1=dstw[:, blk:blk + 1],
                            scalar2=None, op0=Alu.is_equal)
                        if dbg_d is not None and w == 0:
                            sf = spool.tile([P, P], dt.float32, tag="sf")
                            nc.vector.tensor_copy(out=sf[:], in_=S[:])
                            nc.sync.dma_start(out=dbgs_d[:, i, :], in_=sf[:])
                            xf = spool.tile([P, ELEM], dt.float32, tag="xf")
                            nc.vector.tensor_copy(out=xf[:], in_=xga[:, wi * BLKW_A + i, :])
                            nc.sync.dma_start(out=dbgx_d[:, i, :], in_=xf[:])
                        for sl in (slice(0, 160), slice(160, 320)):
                            nc.tensor.matmul(
                                gps[:], S[:], xga[:, wi * BLKW_A + i, sl],
                                start=(mi == 0), stop=(mi == nmm - 1))
                            mi += 1
                    for i in range(BLKW_B):
                        blk = NBLK_A + w * BLKW_B + i
                        S = spool.tile([P, P], dt.bfloat16, tag="S")
                        nc.vector.tensor_scalar(
                            out=S[:], in0=iota[:], scalar1=dstw[:, blk:blk + 1],
                            scalar2=None, op0=Alu.is_equal)
                        if dbg_d is not None and w == 0:
                            sf = spool.tile([P, P], dt.float32, tag="sf")
                            nc.vector.tensor_copy(out=sf[:], in_=S[:])
                            nc.sync.dma_start(out=dbgsb_d[:, i, :], in_=sf[:])
                            xf = spool.tile([P, ELEM], dt.float32, tag="xf")
                            nc.vector.tensor_copy(out=xf[:], in_=xgb[:, wi * BLKW_B + i, :])
                            nc.sync.dma_start(out=dbgxb_d[:, i, :], in_=xf[:])
                        for sl in (slice(0, 160), slice(160, 320)):
                            nc.tensor.matmul(
                                gps[:], S[:], xgb[:, wi * BLKW_B + i, sl],
                                start=(mi == 0), stop=(mi == nmm - 1))
                            mi += 1
                    Gw = trop.tile([P, DIM], dt.float32, tag="Gw")
                    nc.vector.tensor_copy(out=Gw[:], in_=gps[:])
                    if dbg_d is not None and w < 4:
                        nc.sync.dma_start(out=dbg_d[:, w, :], in_=Gw[:])
                    if not has_wmix:
                        nc.vector.tensor_copy(out=obuf[:, w, :], in_=Gw[:])
                        continue
                    # W-mix for window w
                    mps = psum.tile([P, DIM], dt.float32, tag="mps")
                    trp = psum.tile([P, P], dt.float32, tag="trp")
                    nc.tensor.transpose(out=trp[0:SC, :], in_=Gw[:, 0:SC],
                                        identity=ident[:])
                    tr0 = trop.tile([SC, P], dt.float32, tag="tr0")
                    nc.vector.tensor_copy(out=tr0[:], in_=trp[0:SC, :])
                    nc.tensor.matmul(mps[:, 0:SC], tr0[:], w0sb[:],
                                     start=True, stop=True)
                    for j in range(3):
                        trpj = psum.tile([P, P], dt.float32, tag="trpj")
                        nc.tensor.transpose(
                            out=trpj[0:VC, :],
                            in_=Gw[:, SC + VC * j:SC + VC * (j + 1)],
                            identity=ident[:])
                        tr1 = trop.tile([VC, P], dt.float32, tag="tr1")
                        nc.vector.tensor_copy(out=tr1[:], in_=trpj[0:VC, :])
                        nc.tensor.matmul(mps[:, SC + VC * j:SC + VC * (j + 1)],
                                         tr1[:], w1sb[:], start=True, stop=True)
                    nc.vector.tensor_scalar(
                        out=obuf[:, w, 0:SC], in0=mps[:, 0:SC],
                        scalar1=ewc0[:, 0:1], scalar2=None, op0=Alu.mult)
                    nc.vector.tensor_scalar(
                        out=obuf[:, w, SC:DIM], in0=mps[:, SC:DIM],
                        scalar1=ewc1[:, 0:1], scalar2=None, op0=Alu.mult)

            # ---- layernorm + silu on l0 slice (per chunk of windows)
            LNC = 9 if has_wmix else 0

            for c0 in range(0, W, LNC) if LNC else []:
                c1 = min(c0 + LNC, W)
                nw = c1 - c0
                ob0 = obuf[:, c0:c1, 0:SC]
                mub = zpool.tile([P, LNC], dt.float32, tag="mub")
                nc.vector.tensor_reduce(out=mub[:, :nw], in_=ob0,
                                        axis=mybir.AxisListType.X, op=Alu.add)
                mub2 = zpool.tile([P, LNC], dt.float32, tag="mub2")
                nc.vector.tensor_scalar(out=mub2[:, :nw], in0=mub[:, :nw],
                                        scalar1=float(1.0 / SC), scalar2=None,
                                        op0=Alu.mult)
                cen = zpool.tile([P, LNC, SC], dt.float32, tag="cen")
                nc.vector.tensor_tensor(
                    out=cen[:, :nw, :], in0=ob0,
                    in1=mub2[:, :nw].unsqueeze(2).to_broadcast([P, nw, SC]),
                    op=Alu.subtract)
                sqb = zpool.tile([P, LNC, SC], dt.float32, tag="sqb")
                nc.vector.tensor_tensor(out=sqb[:, :nw, :], in0=cen[:, :nw, :],
                                        in1=cen[:, :nw, :], op=Alu.mult)
                varb = zpool.tile([P, LNC], dt.float32, tag="varb")
                nc.vector.tensor_reduce(out=varb[:, :nw], in_=sqb[:, :nw, :],
                                        axis=mybir.AxisListType.X, op=Alu.add)
                vb2 = zpool.tile([P, LNC], dt.float32, tag="vb2")
                nc.vector.tensor_scalar(out=vb2[:, :nw], in0=varb[:, :nw],
                                        scalar1=float(1.0 / SC), scalar2=float(EPS),
                                        op0=Alu.mult, op1=Alu.add)
                sdb = zpool.tile([P, LNC], dt.float32, tag="sdb")
                nc.scalar.activation(out=sdb[:, :nw], in_=vb2[:, :nw], func=Act.Sqrt)
                rsb = zpool.tile([P, LNC], dt.float32, tag="rsb")
                nc.vector.reciprocal(out=rsb[:, :nw], in_=sdb[:, :nw])
                t1b = zpool.tile([P, LNC, SC], dt.float32, tag="lnt")
                nc.vector.tensor_tensor(
                    out=t1b[:, :nw, :], in0=cen[:, :nw, :],
                    in1=rsb[:, :nw].unsqueeze(2).to_broadcast([P, nw, SC]),
                    op=Alu.mult)
                t2b = zpool.tile([P, LNC, SC], dt.float32, tag="lnt")
                nc.vector.tensor_tensor(
                    out=t2b[:, :nw, :], in0=t1b[:, :nw, :],
                    in1=smalls["gam"][:].unsqueeze(1).to_broadcast([P, nw, SC]),
                    op=Alu.mult)
                t3b = zpool.tile([P, LNC, SC], dt.float32, tag="lnt")
                nc.vector.tensor_tensor(
                    out=t3b[:, :nw, :], in0=t2b[:, :nw, :],
                    in1=smalls["bet"][:].unsqueeze(1).to_broadcast([P, nw, SC]),
                    op=Alu.add)
                sgb = zpool.tile([P, LNC, SC], dt.float32, tag="sgb")
                nc.scalar.activation(out=sgb[:, :nw, :], in_=t3b[:, :nw, :],
                                     func=Act.Sigmoid)
                nc.vector.tensor_tensor(out=ob0, in0=t3b[:, :nw, :],
                                        in1=sgb[:, :nw, :], op=Alu.mult)

            # ---- store
            outv = out_d[:].rearrange("(w p) d -> p w d", p=P)
            nchunk = 3
            per = W // nchunk
            for c in range(nchunk):
                wl0 = c * per
                wl1 = W if c == nchunk - 1 else (c + 1) * per
                nc.sync.dma_start(out=outv[:, wl0:wl1, :],
                                  in_=obuf[:, wl0:wl1, :])

    nc.compile()
    return nc


def _get_program():
    global _PROG
    if _PROG is None:
        _PROG = _build_program()
    return _PROG


# ---------------------------------------------------------------- entry point

def kernel(**inputs):
    from concourse.bass_utils import run_bass_kernel_spmd

    x = np.asarray(inputs["x"], f32)
    pos = np.asarray(inputs["pos"], f32)
    ei = np.asarray(inputs["edge_index"])
    src = ei[0].astype(np.int64)
    dst = ei[1].astype(np.int64)

    xp, cores, meta = _stage(x, pos, src, dst)

    iota = np.tile(np.arange(P, dtype=f32)[None, :], (P, 1))
    ident = np.eye(P, dtype=f32)
    common = dict(
        xp=xp, iota=iota, ident=ident,
        w0=np.asarray(inputs["W0"], f32),
        w1=np.asarray(inputs["W1"], f32),
        cent=np.asarray(inputs["rbf_centers"], f32).reshape(1, 64),
        wid=np.asarray(inputs["rbf_widths"], f32).reshape(1, 64),
        pwv=np.asarray(inputs["edge_proj_w"], f32).reshape(1, 64),
        gam=np.asarray(inputs["ln_gamma"], f32).reshape(1, 64),
        bet=np.asarray(inputs["ln_beta"], f32).reshape(1, 64),
        pbv=np.asarray(inputs["edge_proj_b"], f32).reshape(1, 1),
    )
    in_maps = [dict(common, **cores[r]) for r in range(NCORES)]

    nc = _get_program()
    trace = bool(int(os.environ.get("KERNEL_TRACE", "0")))
    res = run_bass_kernel_spmd(nc, in_maps, core_ids=list(range(NCORES)),
                               trace=trace)
    kernel.last_results = res

    # assemble full output
    out_full = np.zeros((N, DIM), f32)
    col_map = np.arange(DIM)
    for jj in range(3):
        for cc in range(VC):
            col_map[SC + 3 * cc + jj] = SC + VC * jj + cc
    n2w, n2s = meta["node2win"], meta["node2slot"]
    wc, wl = meta["win_core"], meta["win_local"]
    for r in range(NCORES):
        o = res.results[r]["out"]                      # [W*P, DIM]
        nodes = np.nonzero(wc == r)[0]
        rows = wl[nodes] * P + n2s[nodes]
        out_full[nodes] = o[rows][:, col_map]
    return out_full

